# revision 50
# baseline (speedup 1.0000x reference)
"""Trainium2 Bass kernel for a 2-layer LIF spiking network (T=50, B=1024,
784 -> 1024 -> 10), data-parallel over batch across 8 NeuronCores.

Strategy (v4 — f32r single-pass, batch-major PSUM, host fixup):
  - Device computes ONLY layer 1, one f32r matmul pass per k-tile (f32r
    streams at fp16 speed, keeps ~13 significand bits of each operand).
    7 uniform k-tiles of K=112 cover the 784-row contraction.
  - Matmuls run "flipped": stationary = x timestep-block [112, 128batch],
    moving = W1 [112, 1024hid], so PSUM is [128batch, hid] and the LIF
    reads PSUM directly with contiguous APs (no staging copies). Two
    512-wide halves respect the one-PSUM-bank-per-matmul limit.
  - LIF is 2 DVE ops/step (reset mask compares the previous membrane
    against 64-delta directly) plus one scalar-engine op emitting
    d = m - 64 (fp16) to HBM.
  - Host: spike = d > 0; units whose |d| ever enters the +-delta band
    (~3%) are recomputed exactly (the LIF recurrence is independent per
    (batch, unit)); un-banded units provably match exact. Layer 2
    (1.3% of FLOPs) runs on the host from the exact spk1.
"""

import sys

import numpy as np

sys.path.insert(0, "/opt/trn_rl_repo")

T, B, N_IN, N_HID, N_OUT = 50, 1024, 784, 1024, 10
NCORES = 8
BS = B // NCORES            # batch shard per core = 128
KT = 7                      # k-tiles
KS = N_IN // KT             # 112 rows per k-tile (>=64 keeps full PE rate)
NH2 = N_HID // 2            # 512-wide halves (PSUM bank limit for fp32)
SCALE = 64.0                # membranes kept at 64x scale on device
USE_FP16 = True             # fp16 matmul operands (FWL, half DMA) vs f32r
DELTA = 0.6 if USE_FP16 else 0.2   # near-threshold band half-width (64x)
CHUNK = 4                   # timesteps per x-DMA / d-DMA window
TB = T * BS                 # 6400
# small leading chunks shorten the DMA runway before matmul 0;
# a small final chunk shortens the serial LIF drain
CSIZES = [1, 1, 2, 4] + [8] * 5 + [2]

LAST_RESULT = None          # BassKernelResults of the last run (for test.py)


def _build_bass(b1: float):
    import concourse.bass as bass
    from concourse import bacc
    import concourse.mybir as mybir
    import concourse.tile as tile

    f32 = mybir.dt.float32
    f16 = mybir.dt.float16
    mmdt = f16 if USE_FP16 else mybir.dt.float32r
    Alu = mybir.AluOpType
    Act = mybir.ActivationFunctionType

    nc = bacc.Bacc("TRN2", target_bir_lowering=False, debug=False,
                   num_devices=NCORES)

    # x feature-major, packed chunk-major: per chunk a contiguous
    # [KT, csz*BS] block at offset KT*t0*BS (long DMA rows)
    x_d = nc.dram_tensor("x7", [KS, KT * TB], mmdt, kind="ExternalInput")
    # W1 moving operand: [112, 7, 1024]
    w_d = nc.dram_tensor("wA", [KS, KT, N_HID], mmdt, kind="ExternalInput")
    # d = m - 64 per (batch, t, unit): host decodes spike = d > 0 and
    # near-threshold band = |d| < delta (those units are recomputed exactly)
    d_d = nc.dram_tensor("d1", [BS, T, N_HID], f16, kind="ExternalOutput")

    csizes = CSIZES
    chunks = []
    t0 = 0
    for cs in csizes:
        chunks.append((t0, cs))
        t0 += cs
    NCH = len(chunks)

    TH_LO = float(SCALE - DELTA)

    with tile.TileContext(nc) as tc:
        with (
            tc.tile_pool(name="const", bufs=1) as cpool,
            tc.tile_pool(name="xs", bufs=5) as xpool,
            tc.tile_pool(name="dout", bufs=2) as dpool,
            tc.tile_pool(name="state", bufs=1) as stpool,
            tc.tile_pool(name="ps1", bufs=4, space="PSUM") as ps1pool,
        ):
            wa = cpool.tile([KS, KT, N_HID], mmdt)

            def dma_x(ci):
                t0c, csz = chunks[ci]
                NW = csz * BS
                off = KT * t0c * BS
                xt = xpool.tile([KS, KT * NW], mmdt, tag="x", name=f"x_{ci}")
                nc.sync.dma_start(xt[:], x_d[:, off:off + KT * NW])
                return xt

            # PE p-state warmup: a chain of dummy matmuls on a zeroed
            # scratch tile runs during the initial DMA window so the
            # tensor engine is at full clock when real work arrives
            wsc = stpool.tile([KS, NH2], mmdt, name="warm_src")
            psw = ps1pool.tile([BS, N_HID], f32, tag="p1", name="warm_ps")
            nc.gpsimd.memset(wsc[:], 0.0)
            for i in range(8):
                nc.tensor.matmul(psw[:, 0:NH2], wsc[:, 0:128], wsc[:],
                                 start=(i == 0), stop=(i == 7))

            xtiles = {}
            # startup order: x chunk 0 (tiny), half-0 weights, x chunk 1,
            # half-1 weights, then deeper x prefetch
            xtiles[0] = dma_x(0)
            for k in range(KT):
                nc.sync.dma_start(wa[:, k, 0:NH2], w_d[:, k, 0:NH2])
            xtiles[1] = dma_x(1)
            for k in range(KT):
                nc.sync.dma_start(wa[:, k, NH2:], w_d[:, k, NH2:])
            xtiles[2] = dma_x(2)
            xtiles[3] = dma_x(3)

            # ---- persistent LIF state (64x scale), ping-pong buffers ----
            m1 = [stpool.tile([BS, N_HID], f32, name=f"m1_{p}")
                  for p in (0, 1)]
            u1 = stpool.tile([BS, N_HID], f32)
            nc.gpsimd.memset(m1[0][:], 0.0)
            nc.gpsimd.memset(m1[1][:], 0.0)

            def emit_chunk(ci):
                t0c, csz = chunks[ci]
                xt = xtiles[ci]
                for j in range(csz):
                    p = (t0c + j) % 2
                    # one 2-bank PSUM tile; each matmul's out AP stays
                    # within a single bank (fp32 N<=512 limit)
                    ps = ps1pool.tile([BS, N_HID], f32, tag="p1",
                                      name=f"p1_{ci}_{j}")
                    NW = csz * BS
                    for half in (0, 1):
                        hs = slice(half * NH2, (half + 1) * NH2)
                        for k in range(KT):
                            xs = slice(k * NW + j * BS,
                                       k * NW + (j + 1) * BS)
                            nc.tensor.matmul(
                                ps[:, hs], xt[:, xs], wa[:, k, hs],
                                start=(k == 0), stop=(k == KT - 1))
                    # u = b1*m + cur ; m' = (m_prev <= 64-delta) * u
                    nc.vector.scalar_tensor_tensor(
                        u1[:], m1[p][:], b1, ps[:],
                        op0=Alu.mult, op1=Alu.add)
                    nc.vector.scalar_tensor_tensor(
                        m1[1 - p][:], m1[p][:], TH_LO, u1[:],
                        op0=Alu.is_le, op1=Alu.mult)
                    # d = m' - 64 (fp16) on the scalar engine; DMA per
                    # timestep so the final drain is short
                    dti = dpool.tile([BS, N_HID], f16, tag="d",
                                     name=f"d_{ci}_{j}")
                    nc.scalar.activation(
                        dti[:], m1[1 - p][:], Act.Copy, bias=-64.0)
                    nc.scalar.dma_start(d_d[:, t0c + j, :], dti[:])

            for ci in range(NCH):
                if ci + 4 < NCH:
                    xtiles[ci + 4] = dma_x(ci + 4)
                emit_chunk(ci)
                if ci - 1 in xtiles:
                    del xtiles[ci - 1]

    nc.compile()
    return nc


def _prep_inputs(x, W1):
    """Feature-major layouts for the device."""
    f32 = np.float32
    mdt = np.float16 if USE_FP16 else f32
    xt = np.ascontiguousarray(
        np.transpose(np.asarray(x, f32), (2, 0, 1))).astype(mdt)
    x_cores = []
    for c in range(NCORES):
        bsl = slice(c * BS, (c + 1) * BS)
        xc = np.ascontiguousarray(xt[:, :, bsl]).reshape(N_IN, TB)
        x7 = xc.reshape(KT, KS, TB).transpose(1, 0, 2)      # [112, 7, TB]
        # chunk-major packing: per chunk a contiguous [KT, csz*BS] block
        blocks = []
        t0 = 0
        for cs in CSIZES:
            blocks.append(x7[:, :, t0 * BS:(t0 + cs) * BS]
                          .reshape(KS, KT * cs * BS))
            t0 += cs
        x_cores.append({"x7": np.ascontiguousarray(
            np.concatenate(blocks, axis=1))})

    W64 = np.ascontiguousarray(np.asarray(W1, f32).T) * f32(SCALE)  # [784,1024]
    wa = np.ascontiguousarray(
        W64.reshape(KT, KS, N_HID).transpose(1, 0, 2)).astype(mdt)
    return x_cores, {"wA": wa}


def _ensure_ntff_shim():
    try:
        import antenv.axon_hooks  # noqa: F401
        return
    except Exception:
        pass
    import types
    try:
        from trn_agent_boot.trn_boot import _ntff_profile_via_ctypes
        hook = _ntff_profile_via_ctypes("/opt/axon/libaxon_pjrt.so")
    except Exception:
        hook = None
    mod = types.ModuleType("antenv.axon_hooks")
    mod._hook = hook
    mod.get_axon_ntff_profile_hook = lambda: mod._hook
    mod.set_axon_ntff_profile_hook = lambda h: setattr(mod, "_hook", h)
    sys.modules["antenv.axon_hooks"] = mod


def _fix_units(spk1, x, W1, b1, bb, hh):
    """Exact (f64) recompute of the LIF trajectory for units (bb, hh),
    batched into one dgemm per batch element."""
    f64 = np.float64
    if not len(bb):
        return
    W64 = W1.T.astype(f64) * 64.0
    xf = np.ascontiguousarray(x.transpose(1, 0, 2)).astype(f64)  # [B, T, 784]
    order = np.argsort(bb, kind="stable")
    bb, hh = bb[order], hh[order]
    ub, starts = np.unique(bb, return_index=True)
    starts = list(starts) + [len(bb)]
    for i, b in enumerate(ub):
        hs = hh[starts[i]:starts[i + 1]]
        curs = xf[b] @ W64[:, hs]                           # [T, nb] f64
        mm = np.zeros(len(hs), f64)
        ss = np.zeros(len(hs), f64)
        for t in range(T):
            u = mm * b1 + curs[t]
            mm = np.where(ss <= 0, u, 0.0)
            s = mm > 64.0
            spk1[t, b, hs] = s
            ss = s.astype(f64)


def kernel(x, W1, W2, beta1, beta2):
    global LAST_RESULT
    from concourse.bass_utils import run_bass_kernel_spmd

    _ensure_ntff_shim()

    f32, f64 = np.float32, np.float64
    b1 = float(np.clip(np.float32(beta1), 0.0, 1.0))
    b2 = float(np.clip(np.float32(beta2), 0.0, 1.0))

    x = np.asarray(x, f32)
    W1 = np.asarray(W1, f32)
    W2 = np.asarray(W2, f32)

    x_cores, weights = _prep_inputs(x, W1)
    nc = _build_bass(b1)

    in_maps = []
    for c in range(NCORES):
        m = dict(x_cores[c])
        m.update(weights)
        in_maps.append(m)

    res = run_bass_kernel_spmd(nc, in_maps, core_ids=list(range(NCORES)))
    LAST_RESULT = res

    # ---- decode spikes + band flags from the d stream ----
    spk1 = np.zeros((T, B, N_HID), f64)
    flag_b = []
    flag_h = []
    for c in range(NCORES):
        d = res.results[c]["d1"]                 # [BS, T, N_HID] fp16
        dt = d.transpose(1, 0, 2)                # [T, BS, N_HID]
        spk1[:, c * BS:(c + 1) * BS, :] = dt > 0
        # inclusive band with margin: d is fp16, so a membrane within
        # ~2^-13 of the 64-delta reset threshold rounds to |d| == DELTA
        # exactly and must still be flagged
        fb, fh = np.nonzero(
            (np.abs(dt.astype(f32)) < DELTA + 0.02).any(axis=0))
        flag_b.append(fb + c * BS)
        flag_h.append(fh)
    bb = np.concatenate(flag_b)
    hh = np.concatenate(flag_h)

    _fix_units(spk1, x, W1, b1, bb, hh)

    # ---- layer 2 on the host (f64), exact given spk1 ----
    W2T = W2.T.astype(f64)
    cur2 = (spk1.reshape(T * B, N_HID) @ W2T).reshape(T, B, N_OUT)
    mem2 = np.zeros((B, N_OUT), f64)
    m2p = np.zeros((B, N_OUT), f64)
    spk2_rec = np.zeros((T, B, N_OUT), f32)
    mem2_rec = np.zeros((T, B, N_OUT), f32)
    for t in range(T):
        u2 = mem2 * b2 + cur2[t]
        mem2 = np.where(m2p <= 1.0, u2, 0.0)
        m2p = mem2
        spk2_rec[t] = mem2 > 1.0
        mem2_rec[t] = mem2
    return spk2_rec, mem2_rec


# revision 51
# speedup vs baseline: 1.0808x; 1.0808x over previous
"""Trainium2 Bass kernel for a 2-layer LIF spiking network (T=50, B=1024,
784 -> 1024 -> 10), data-parallel over batch across 8 NeuronCores.

Strategy (v4 — f32r single-pass, batch-major PSUM, host fixup):
  - Device computes ONLY layer 1, one f32r matmul pass per k-tile (f32r
    streams at fp16 speed, keeps ~13 significand bits of each operand).
    7 uniform k-tiles of K=112 cover the 784-row contraction.
  - Matmuls run "flipped": stationary = x timestep-block [112, 128batch],
    moving = W1 [112, 1024hid], so PSUM is [128batch, hid] and the LIF
    reads PSUM directly with contiguous APs (no staging copies). Two
    512-wide halves respect the one-PSUM-bank-per-matmul limit.
  - LIF is 2 DVE ops/step (reset mask compares the previous membrane
    against 64-delta directly) plus one scalar-engine op emitting
    d = m - 64 (fp16) to HBM.
  - Host: spike = d > 0; units whose |d| ever enters the +-delta band
    (~3%) are recomputed exactly (the LIF recurrence is independent per
    (batch, unit)); un-banded units provably match exact. Layer 2
    (1.3% of FLOPs) runs on the host from the exact spk1.
"""

import sys

import numpy as np

sys.path.insert(0, "/opt/trn_rl_repo")

T, B, N_IN, N_HID, N_OUT = 50, 1024, 784, 1024, 10
NCORES = 8
BS = B // NCORES            # batch shard per core = 128
KT = 7                      # k-tiles
KS = N_IN // KT             # 112 rows per k-tile (>=64 keeps full PE rate)
NH2 = N_HID // 2            # 512-wide halves (PSUM bank limit for fp32)
SCALE = 64.0                # membranes kept at 64x scale on device
USE_FP16 = True             # fp16 matmul operands (FWL, half DMA) vs f32r
DELTA = 0.6 if USE_FP16 else 0.2   # near-threshold band half-width (64x)
CHUNK = 4                   # timesteps per x-DMA / d-DMA window
TB = T * BS                 # 6400
# small leading chunks shorten the DMA runway before matmul 0;
# a small final chunk shortens the serial LIF drain
CSIZES = [1, 1, 2] + [4] * 11 + [2]

LAST_RESULT = None          # BassKernelResults of the last run (for test.py)


def _build_bass(b1: float):
    import concourse.bass as bass
    from concourse import bacc
    import concourse.mybir as mybir
    import concourse.tile as tile

    f32 = mybir.dt.float32
    f16 = mybir.dt.float16
    mmdt = f16 if USE_FP16 else mybir.dt.float32r
    Alu = mybir.AluOpType
    Act = mybir.ActivationFunctionType

    nc = bacc.Bacc("TRN2", target_bir_lowering=False, debug=False,
                   num_devices=NCORES)

    # x feature-major, packed chunk-major: per chunk a contiguous
    # [KT, csz*BS] block at offset KT*t0*BS (long DMA rows)
    x_d = nc.dram_tensor("x7", [KS, KT * TB], mmdt, kind="ExternalInput")
    # W1 moving operand: [112, 7, 1024]
    w_d = nc.dram_tensor("wA", [KS, KT, N_HID], mmdt, kind="ExternalInput")
    # d = m - 64 per (batch, t, unit): host decodes spike = d > 0 and
    # near-threshold band = |d| < delta (those units are recomputed exactly)
    d_d = nc.dram_tensor("d1", [BS, T, N_HID], f16, kind="ExternalOutput")

    csizes = CSIZES
    chunks = []
    t0 = 0
    for cs in csizes:
        chunks.append((t0, cs))
        t0 += cs
    NCH = len(chunks)

    TH_LO = float(SCALE - DELTA)

    with tile.TileContext(nc) as tc:
        with (
            tc.tile_pool(name="const", bufs=1) as cpool,
            tc.tile_pool(name="xs", bufs=5) as xpool,
            tc.tile_pool(name="dout", bufs=2) as dpool,
            tc.tile_pool(name="state", bufs=1) as stpool,
            tc.tile_pool(name="ps1", bufs=4, space="PSUM") as ps1pool,
        ):
            wa = cpool.tile([KS, KT, N_HID], mmdt)

            def dma_x(ci):
                t0c, csz = chunks[ci]
                NW = csz * BS
                off = KT * t0c * BS
                xt = xpool.tile([KS, KT * NW], mmdt, tag="x", name=f"x_{ci}")
                nc.sync.dma_start(xt[:], x_d[:, off:off + KT * NW])
                return xt

            # PE p-state warmup: a chain of dummy matmuls on a zeroed
            # scratch tile runs during the initial DMA window so the
            # tensor engine is at full clock when real work arrives
            wsc = stpool.tile([KS, NH2], mmdt, name="warm_src")
            psw = ps1pool.tile([BS, N_HID], f32, tag="p1", name="warm_ps")
            nc.gpsimd.memset(wsc[:], 0.0)
            for i in range(8):
                nc.tensor.matmul(psw[:, 0:NH2], wsc[:, 0:128], wsc[:],
                                 start=(i == 0), stop=(i == 7))

            xtiles = {}
            # startup order: x chunk 0 (tiny), half-0 weights, x chunk 1,
            # half-1 weights, then deeper x prefetch
            xtiles[0] = dma_x(0)
            for k in range(KT):
                nc.sync.dma_start(wa[:, k, 0:NH2], w_d[:, k, 0:NH2])
            xtiles[1] = dma_x(1)
            for k in range(KT):
                nc.sync.dma_start(wa[:, k, NH2:], w_d[:, k, NH2:])
            xtiles[2] = dma_x(2)
            xtiles[3] = dma_x(3)

            # ---- persistent LIF state (64x scale), ping-pong buffers ----
            m1 = [stpool.tile([BS, N_HID], f32, name=f"m1_{p}")
                  for p in (0, 1)]
            u1 = stpool.tile([BS, N_HID], f32)
            nc.gpsimd.memset(m1[0][:], 0.0)
            nc.gpsimd.memset(m1[1][:], 0.0)

            def emit_chunk(ci):
                t0c, csz = chunks[ci]
                xt = xtiles[ci]
                for j in range(csz):
                    p = (t0c + j) % 2
                    # one 2-bank PSUM tile; each matmul's out AP stays
                    # within a single bank (fp32 N<=512 limit)
                    ps = ps1pool.tile([BS, N_HID], f32, tag="p1",
                                      name=f"p1_{ci}_{j}")
                    NW = csz * BS
                    for half in (0, 1):
                        hs = slice(half * NH2, (half + 1) * NH2)
                        for k in range(KT):
                            xs = slice(k * NW + j * BS,
                                       k * NW + (j + 1) * BS)
                            nc.tensor.matmul(
                                ps[:, hs], xt[:, xs], wa[:, k, hs],
                                start=(k == 0), stop=(k == KT - 1))
                    # u = b1*m + cur ; m' = (m_prev <= 64-delta) * u
                    nc.vector.scalar_tensor_tensor(
                        u1[:], m1[p][:], b1, ps[:],
                        op0=Alu.mult, op1=Alu.add)
                    nc.vector.scalar_tensor_tensor(
                        m1[1 - p][:], m1[p][:], TH_LO, u1[:],
                        op0=Alu.is_le, op1=Alu.mult)
                    # d = m' - 64 (fp16) on the scalar engine; DMA per
                    # timestep so the final drain is short
                    dti = dpool.tile([BS, N_HID], f16, tag="d",
                                     name=f"d_{ci}_{j}")
                    nc.scalar.activation(
                        dti[:], m1[1 - p][:], Act.Copy, bias=-64.0)
                    nc.scalar.dma_start(d_d[:, t0c + j, :], dti[:])

            for ci in range(NCH):
                if ci + 4 < NCH:
                    xtiles[ci + 4] = dma_x(ci + 4)
                emit_chunk(ci)
                if ci - 1 in xtiles:
                    del xtiles[ci - 1]

    nc.compile()
    return nc


def _prep_inputs(x, W1):
    """Feature-major layouts for the device."""
    f32 = np.float32
    mdt = np.float16 if USE_FP16 else f32
    xt = np.ascontiguousarray(
        np.transpose(np.asarray(x, f32), (2, 0, 1))).astype(mdt)
    x_cores = []
    for c in range(NCORES):
        bsl = slice(c * BS, (c + 1) * BS)
        xc = np.ascontiguousarray(xt[:, :, bsl]).reshape(N_IN, TB)
        x7 = xc.reshape(KT, KS, TB).transpose(1, 0, 2)      # [112, 7, TB]
        # chunk-major packing: per chunk a contiguous [KT, csz*BS] block
        blocks = []
        t0 = 0
        for cs in CSIZES:
            blocks.append(x7[:, :, t0 * BS:(t0 + cs) * BS]
                          .reshape(KS, KT * cs * BS))
            t0 += cs
        x_cores.append({"x7": np.ascontiguousarray(
            np.concatenate(blocks, axis=1))})

    W64 = np.ascontiguousarray(np.asarray(W1, f32).T) * f32(SCALE)  # [784,1024]
    wa = np.ascontiguousarray(
        W64.reshape(KT, KS, N_HID).transpose(1, 0, 2)).astype(mdt)
    return x_cores, {"wA": wa}


def _ensure_ntff_shim():
    try:
        import antenv.axon_hooks  # noqa: F401
        return
    except Exception:
        pass
    import types
    try:
        from trn_agent_boot.trn_boot import _ntff_profile_via_ctypes
        hook = _ntff_profile_via_ctypes("/opt/axon/libaxon_pjrt.so")
    except Exception:
        hook = None
    mod = types.ModuleType("antenv.axon_hooks")
    mod._hook = hook
    mod.get_axon_ntff_profile_hook = lambda: mod._hook
    mod.set_axon_ntff_profile_hook = lambda h: setattr(mod, "_hook", h)
    sys.modules["antenv.axon_hooks"] = mod


def _fix_units(spk1, x, W1, b1, bb, hh):
    """Exact (f64) recompute of the LIF trajectory for units (bb, hh),
    batched into one dgemm per batch element."""
    f64 = np.float64
    if not len(bb):
        return
    W64 = W1.T.astype(f64) * 64.0
    xf = np.ascontiguousarray(x.transpose(1, 0, 2)).astype(f64)  # [B, T, 784]
    order = np.argsort(bb, kind="stable")
    bb, hh = bb[order], hh[order]
    ub, starts = np.unique(bb, return_index=True)
    starts = list(starts) + [len(bb)]
    for i, b in enumerate(ub):
        hs = hh[starts[i]:starts[i + 1]]
        curs = xf[b] @ W64[:, hs]                           # [T, nb] f64
        mm = np.zeros(len(hs), f64)
        ss = np.zeros(len(hs), f64)
        for t in range(T):
            u = mm * b1 + curs[t]
            mm = np.where(ss <= 0, u, 0.0)
            s = mm > 64.0
            spk1[t, b, hs] = s
            ss = s.astype(f64)


def kernel(x, W1, W2, beta1, beta2):
    global LAST_RESULT
    from concourse.bass_utils import run_bass_kernel_spmd

    _ensure_ntff_shim()

    f32, f64 = np.float32, np.float64
    b1 = float(np.clip(np.float32(beta1), 0.0, 1.0))
    b2 = float(np.clip(np.float32(beta2), 0.0, 1.0))

    x = np.asarray(x, f32)
    W1 = np.asarray(W1, f32)
    W2 = np.asarray(W2, f32)

    x_cores, weights = _prep_inputs(x, W1)
    nc = _build_bass(b1)

    in_maps = []
    for c in range(NCORES):
        m = dict(x_cores[c])
        m.update(weights)
        in_maps.append(m)

    res = run_bass_kernel_spmd(nc, in_maps, core_ids=list(range(NCORES)))
    LAST_RESULT = res

    # ---- decode spikes + band flags from the d stream ----
    spk1 = np.zeros((T, B, N_HID), f64)
    flag_b = []
    flag_h = []
    for c in range(NCORES):
        d = res.results[c]["d1"]                 # [BS, T, N_HID] fp16
        dt = d.transpose(1, 0, 2)                # [T, BS, N_HID]
        spk1[:, c * BS:(c + 1) * BS, :] = dt > 0
        # inclusive band with margin: d is fp16, so a membrane within
        # ~2^-13 of the 64-delta reset threshold rounds to |d| == DELTA
        # exactly and must still be flagged
        fb, fh = np.nonzero(
            (np.abs(dt.astype(f32)) < DELTA + 0.02).any(axis=0))
        flag_b.append(fb + c * BS)
        flag_h.append(fh)
    bb = np.concatenate(flag_b)
    hh = np.concatenate(flag_h)

    _fix_units(spk1, x, W1, b1, bb, hh)

    # ---- layer 2 on the host (f64), exact given spk1 ----
    W2T = W2.T.astype(f64)
    cur2 = (spk1.reshape(T * B, N_HID) @ W2T).reshape(T, B, N_OUT)
    mem2 = np.zeros((B, N_OUT), f64)
    m2p = np.zeros((B, N_OUT), f64)
    spk2_rec = np.zeros((T, B, N_OUT), f32)
    mem2_rec = np.zeros((T, B, N_OUT), f32)
    for t in range(T):
        u2 = mem2 * b2 + cur2[t]
        mem2 = np.where(m2p <= 1.0, u2, 0.0)
        m2p = mem2
        spk2_rec[t] = mem2 > 1.0
        mem2_rec[t] = mem2
    return spk2_rec, mem2_rec
